# revision 6
# baseline (speedup 1.0000x reference)
"""BitLinear (per-token int8 activation quant + ternary weight quant + matmul)
as a Bass/Tile kernel on 8 Trainium2 NeuronCores.

Strategy (tensor-parallel over out_features):
  - weight [4096,4096]; core i ternarizes rows [512i, 512(i+1)) and KEEPS the
    transposed ternary slab resident in SBUF -- no weight exchange at all.
  - x [4,2048,4096] -> [8192,4096]; every core streams ALL tokens, quantizes
    them (per-token scales shared via a tiny amax AllGather), and computes
    out[:, 512i:512(i+1)] = q @ tw_i^T * dq. The host assembles the column
    slabs -- no output collective either.
  - Only two tiny collectives (amax AllGather 4KB, |W|-sum AllReduce 512B).
  - q = rint(x*s) and tw in {-1,0,1} are exact in bf16 => bf16 matmul with
    fp32 PSUM accumulation is EXACT integer arithmetic.
  - The quant -> transpose -> matmul -> dequant chain is software-pipelined
    two token-tiles ahead so every engine's in-order stream runs stall-free.
"""
import numpy as np
from contextlib import ExitStack

N_CORES = 8
B, S, D_IN, D_OUT = 4, 2048, 4096, 4096
TOK = B * S                  # 8192
TOK_PC = TOK // N_CORES      # 1024 tokens quantized-for-amax per core
OF_PC = D_OUT // N_CORES     # 512 out-features per core
N_TT = TOK // 128            # 64 token tiles (all tokens, every core)
N_OWN = TOK_PC // 128        # 8 own token tiles
N_K = D_IN // 128            # 32 contraction tiles
EPS = 1e-5
MAGIC = float(np.float32(1.5 * 2 ** 23))   # fp32 round-to-nearest-even trick
MEAN_SCALE = float(np.float32(1.0 / (D_IN * D_OUT)))  # 2^-24, exact
LOOKAHEAD = 2

_CACHE = {}


def _build_module():
    import concourse.bacc as bacc
    import concourse.tile as tile
    import concourse.mybir as mybir
    import concourse.bass_isa as bass_isa

    dt = mybir.dt
    AF = mybir.ActivationFunctionType
    AL = mybir.AluOpType
    AX = mybir.AxisListType

    nc = bacc.Bacc(
        "TRN2", target_bir_lowering=False, debug=False, num_devices=N_CORES
    )
    xf = nc.dram_tensor("xf", [TOK, D_IN], dt.float32, kind="ExternalInput").ap()
    xo = nc.dram_tensor("xo", [TOK_PC, D_IN], dt.float32, kind="ExternalInput").ap()
    ws = nc.dram_tensor("ws", [OF_PC, D_IN], dt.float32, kind="ExternalInput").ap()
    out = nc.dram_tensor("out", [TOK, OF_PC], dt.float32, kind="ExternalOutput").ap()

    am_d = nc.dram_tensor("am_d", [128, N_OWN], dt.float32).ap()
    am_sh = nc.dram_tensor(
        "am_sh", [N_CORES * 128, N_OWN], dt.float32, addr_space="Shared"
    ).ap()
    wsum_d = nc.dram_tensor("wsum_d", [128, 1], dt.float32).ap()
    wsum_sh = nc.dram_tensor("wsum_sh", [128, 1], dt.float32, addr_space="Shared").ap()

    NWT = OF_PC // 128  # 4 weight row-blocks per core

    with tile.TileContext(nc) as tc, ExitStack() as ctx:
        stats = ctx.enter_context(tc.tile_pool(name="stats", bufs=1))
        xpool = ctx.enter_context(tc.tile_pool(name="xpool", bufs=3))
        qp = ctx.enter_context(tc.tile_pool(name="qp", bufs=2))
        qTp = ctx.enter_context(tc.tile_pool(name="qTp", bufs=4))
        twTo = ctx.enter_context(tc.tile_pool(name="twTo", bufs=1))
        op = ctx.enter_context(tc.tile_pool(name="op", bufs=3))
        pp = ctx.enter_context(tc.tile_pool(name="pp", bufs=6, space="PSUM"))

        am_own = stats.tile([128, N_OWN], dt.float32, tag="am_own")
        amg = stats.tile([128, N_TT], dt.float32, tag="amg")
        s_all = stats.tile([128, N_TT], dt.float32, tag="s_all")
        dq = stats.tile([128, N_TT], dt.float32, tag="dq")
        wme = stats.tile([128, 1], dt.float32, tag="wme")
        swt = stats.tile([128, 1], dt.float32, tag="swt")
        wp = stats.tile([128, NWT], dt.float32, tag="wp")
        wsum_sb = stats.tile([128, 1], dt.float32, tag="wsum_sb")
        gsb = stats.tile([128, 1], dt.float32, tag="gsb")
        gtot = stats.tile([128, 1], dt.float32, tag="gtot")

        # ---- section 1: own-token amax (gates the amax AllGather) ----
        with nc.named_scope("amax"), tc.tile_pool(name="xop", bufs=2) as xop:
            for t in range(N_OWN):
                xot = xop.tile([128, D_IN], dt.float32, tag="xo", name=f"xot{t}")
                nc.scalar.dma_start(xot[:], xo[t * 128:(t + 1) * 128, :])
                nc.vector.tensor_reduce(
                    am_own[:, t:t + 1], xot[:], axis=AX.X, op=AL.max,
                    apply_absolute_value=True,
                )
            nc.scalar.dma_start(am_d[:], am_own[:])

        # ---- section 2: |W| partial sums (gates the AllReduce) ----
        with nc.named_scope("wsum"), tc.tile_pool(name="wpool", bufs=3) as wpool:
            for j in range(NWT):
                wt = wpool.tile([128, D_IN], dt.float32, tag="w", name=f"wt{j}")
                nc.scalar.dma_start(wt[:], ws[j * 128:(j + 1) * 128, :])
                nc.vector.tensor_reduce(
                    wp[:, j:j + 1], wt[:], axis=AX.X, op=AL.add,
                    apply_absolute_value=True,
                )
            nc.vector.tensor_reduce(wsum_sb[:], wp[:], axis=AX.X, op=AL.add)
            nc.scalar.dma_start(wsum_d[:], wsum_sb[:])

            # ---- section 3: collectives + global scales ----
            with nc.named_scope("coll"):
                nc.gpsimd.collective_compute(
                    "AllGather", AL.bypass,
                    replica_groups=[list(range(N_CORES))],
                    ins=[am_d[:]], outs=[am_sh[:]],
                )
                nc.gpsimd.collective_compute(
                    "AllReduce", AL.add,
                    replica_groups=[list(range(N_CORES))],
                    ins=[wsum_d[:]], outs=[wsum_sh[:]],
                )
                for b in range(N_CORES):
                    nc.sync.dma_start(
                        amg[:, b * N_OWN:(b + 1) * N_OWN],
                        am_sh[b * 128:(b + 1) * 128, :],
                    )
                nc.sync.dma_start(gsb[:], wsum_sh[:])
                nc.gpsimd.partition_all_reduce(
                    gtot[:], gsb[:], channels=128, reduce_op=bass_isa.ReduceOp.add
                )
                nc.vector.tensor_scalar(
                    wme[:], gtot[:], MEAN_SCALE, EPS, op0=AL.mult, op1=AL.max
                )
                nc.vector.reciprocal(swt[:], wme[:])
                # s = 127/max(amax, EPS); dq = max(amax,EPS) * wme / 127
                nc.vector.tensor_scalar(amg[:], amg[:], EPS, None, op0=AL.max)
                nc.vector.reciprocal(s_all[:], amg[:])
                nc.vector.tensor_scalar(s_all[:], s_all[:], 127.0, None, op0=AL.mult)
                nc.vector.tensor_scalar(
                    dq[:], amg[:], wme[:, 0:1],
                    float(np.float32(1.0 / 127.0)), op0=AL.mult, op1=AL.mult,
                )

            # ---- section 4: ternarize own slab + local transpose (resident) ----
            twT_own = twTo.tile([128, N_K, OF_PC], dt.bfloat16, tag="twTo")
            with nc.named_scope("terniarize"):
                for j in range(NWT):
                    wt2 = wpool.tile([128, D_IN], dt.float32, tag="w", name=f"wt2_{j}")
                    nc.scalar.dma_start(wt2[:], ws[j * 128:(j + 1) * 128, :])
                    nc.scalar.activation(wt2[:], wt2[:], AF.Copy, scale=swt[:, 0:1])
                    twr = qp.tile([128, D_IN], dt.bfloat16, tag="qb", name=f"twr{j}")
                    nc.vector.tensor_scalar(
                        twr[:], wt2[:], MAGIC, MAGIC, op0=AL.add, op1=AL.subtract
                    )
                    twc = qp.tile([128, D_IN], dt.bfloat16, tag="qb", name=f"twc{j}")
                    nc.vector.tensor_scalar(
                        twc[:], twr[:], 1.0, -1.0, op0=AL.min, op1=AL.max
                    )
                    nc.sync.dma_start(
                        twT_own[:, :, j * 128:(j + 1) * 128], twc[:], transpose=True
                    )

        # ---- section 5: pipelined quant + matmul over all 64 token tiles ----
        qT_tiles = [None] * N_TT

        def stage_quant(t):
            xt = xpool.tile([128, D_IN], dt.float32, tag="x", name=f"xt{t}")
            nc.scalar.dma_start(xt[:], xf[t * 128:(t + 1) * 128, :])
            nc.scalar.activation(xt[:], xt[:], AF.Copy, scale=s_all[:, t:t + 1])
            qb = qp.tile([128, D_IN], dt.bfloat16, tag="qb", name=f"qb{t}")
            nc.vector.tensor_scalar(
                qb[:], xt[:], MAGIC, MAGIC, op0=AL.add, op1=AL.subtract
            )
            qT_t = qTp.tile([128, N_K, 128], dt.bfloat16, tag="qT", name=f"qT{t}")
            nc.sync.dma_start(qT_t[:], qb[:], transpose=True)
            qT_tiles[t] = qT_t

        def stage_mm(t):
            ps = pp.tile([128, OF_PC], dt.float32, tag="ps", name=f"ps{t}")
            for k in range(N_K):
                nc.tensor.matmul(
                    ps[:], qT_tiles[t][:, k, :], twT_own[:, k, :],
                    start=(k == 0), stop=(k == N_K - 1),
                )
            ot = op.tile([128, OF_PC], dt.float32, tag="ot", name=f"ot{t}")
            nc.scalar.mul(ot[:], ps[:], dq[:, t:t + 1])
            nc.gpsimd.dma_start(out[t * 128:(t + 1) * 128, :], ot[:])

        with nc.named_scope("matmul"):
            for t in range(LOOKAHEAD):
                stage_quant(t)
            for t in range(N_TT):
                if t + LOOKAHEAD < N_TT:
                    stage_quant(t + LOOKAHEAD)
                stage_mm(t)

    nc.compile()
    return nc


def _get_module():
    if "nc" not in _CACHE:
        _CACHE["nc"] = _build_module()
    return _CACHE["nc"]


def _make_in_maps(x2, w2):
    return [
        {
            "xf": x2,
            "xo": x2[i * TOK_PC:(i + 1) * TOK_PC],
            "ws": w2[i * OF_PC:(i + 1) * OF_PC],
        }
        for i in range(N_CORES)
    ]


def kernel(x: np.ndarray, weight: np.ndarray) -> np.ndarray:
    from concourse.bass_utils import run_bass_kernel_spmd

    x = np.asarray(x, dtype=np.float32)
    weight = np.asarray(weight, dtype=np.float32)
    x2 = np.ascontiguousarray(x.reshape(TOK, D_IN))
    w2 = np.ascontiguousarray(weight)

    in_maps = _make_in_maps(x2, w2)
    nc = _get_module()
    res = run_bass_kernel_spmd(nc, in_maps, list(range(N_CORES)))
    full = np.empty((TOK, D_OUT), dtype=np.float32)
    for i in range(N_CORES):
        full[:, i * OF_PC:(i + 1) * OF_PC] = res.results[i]["out"]
    return full.reshape(B, S, D_OUT)


# revision 7
# speedup vs baseline: 1.0441x; 1.0441x over previous
"""BitLinear (per-token int8 activation quant + ternary weight quant + matmul)
as a Bass/Tile kernel on 8 Trainium2 NeuronCores.

Strategy (tensor-parallel over out_features):
  - weight [4096,4096]; core i ternarizes rows [512i, 512(i+1)) and KEEPS the
    transposed ternary slab resident in SBUF -- no weight exchange at all.
  - x [4,2048,4096] -> [8192,4096]; every core streams ALL tokens, quantizes
    them (per-token scales shared via a tiny amax AllGather), and computes
    out[:, 512i:512(i+1)] = q @ tw_i^T * dq. The host assembles the column
    slabs -- no output collective either.
  - Only two tiny collectives (amax AllGather 4KB, |W|-sum AllReduce 512B).
  - q = rint(x*s) and tw in {-1,0,1} are exact in bf16 => bf16 matmul with
    fp32 PSUM accumulation is EXACT integer arithmetic.
  - The quant -> transpose -> matmul -> dequant chain is software-pipelined
    two token-tiles ahead so every engine's in-order stream runs stall-free.
"""
import numpy as np
from contextlib import ExitStack

N_CORES = 8
B, S, D_IN, D_OUT = 4, 2048, 4096, 4096
TOK = B * S                  # 8192
TOK_PC = TOK // N_CORES      # 1024 tokens quantized-for-amax per core
OF_PC = D_OUT // N_CORES     # 512 out-features per core
N_TT = TOK // 128            # 64 token tiles (all tokens, every core)
N_OWN = TOK_PC // 128        # 8 own token tiles
N_K = D_IN // 128            # 32 contraction tiles
EPS = 1e-5
MAGIC = float(np.float32(1.5 * 2 ** 23))   # fp32 round-to-nearest-even trick
MEAN_SCALE = float(np.float32(1.0 / (D_IN * D_OUT)))  # 2^-24, exact
LOOKAHEAD = 5

_CACHE = {}


def _build_module():
    import concourse.bacc as bacc
    import concourse.tile as tile
    import concourse.mybir as mybir
    import concourse.bass_isa as bass_isa

    dt = mybir.dt
    AF = mybir.ActivationFunctionType
    AL = mybir.AluOpType
    AX = mybir.AxisListType

    nc = bacc.Bacc(
        "TRN2", target_bir_lowering=False, debug=False, num_devices=N_CORES
    )
    xf = nc.dram_tensor("xf", [TOK, D_IN], dt.float32, kind="ExternalInput").ap()
    xo = nc.dram_tensor("xo", [TOK_PC, D_IN], dt.float32, kind="ExternalInput").ap()
    ws = nc.dram_tensor("ws", [OF_PC, D_IN], dt.float32, kind="ExternalInput").ap()
    out = nc.dram_tensor("out", [TOK, OF_PC], dt.float32, kind="ExternalOutput").ap()

    am_d = nc.dram_tensor("am_d", [128, N_OWN], dt.float32).ap()
    am_sh = nc.dram_tensor(
        "am_sh", [N_CORES * 128, N_OWN], dt.float32, addr_space="Shared"
    ).ap()
    wsum_d = nc.dram_tensor("wsum_d", [128, 1], dt.float32).ap()
    wsum_sh = nc.dram_tensor("wsum_sh", [128, 1], dt.float32, addr_space="Shared").ap()

    NWT = OF_PC // 128  # 4 weight row-blocks per core

    with tile.TileContext(nc) as tc, ExitStack() as ctx:
        stats = ctx.enter_context(tc.tile_pool(name="stats", bufs=1))
        xpool = ctx.enter_context(tc.tile_pool(name="xpool", bufs=4))
        qp = ctx.enter_context(tc.tile_pool(name="qp", bufs=3))
        qTp = ctx.enter_context(tc.tile_pool(name="qTp", bufs=6))
        twTo = ctx.enter_context(tc.tile_pool(name="twTo", bufs=1))
        op = ctx.enter_context(tc.tile_pool(name="op", bufs=3))
        pp = ctx.enter_context(tc.tile_pool(name="pp", bufs=6, space="PSUM"))

        am_own = stats.tile([128, N_OWN], dt.float32, tag="am_own")
        amg = stats.tile([128, N_TT], dt.float32, tag="amg")
        s_all = stats.tile([128, N_TT], dt.float32, tag="s_all")
        dq = stats.tile([128, N_TT], dt.float32, tag="dq")
        wme = stats.tile([128, 1], dt.float32, tag="wme")
        swt = stats.tile([128, 1], dt.float32, tag="swt")
        wp = stats.tile([128, NWT], dt.float32, tag="wp")
        wsum_sb = stats.tile([128, 1], dt.float32, tag="wsum_sb")
        gsb = stats.tile([128, 1], dt.float32, tag="gsb")
        gtot = stats.tile([128, 1], dt.float32, tag="gtot")

        # ---- section 1: own-token amax (gates the amax AllGather) ----
        with nc.named_scope("amax"):
            for t in range(N_OWN):
                xot = xpool.tile([128, D_IN], dt.float32, tag="x", name=f"xot{t}")
                nc.scalar.dma_start(xot[:], xo[t * 128:(t + 1) * 128, :])
                nc.vector.tensor_reduce(
                    am_own[:, t:t + 1], xot[:], axis=AX.X, op=AL.max,
                    apply_absolute_value=True,
                )
            nc.scalar.dma_start(am_d[:], am_own[:])

        # ---- section 2: |W| partial sums (gates the AllReduce) ----
        with nc.named_scope("wsum"):
            for j in range(NWT):
                wt = xpool.tile([128, D_IN], dt.float32, tag="x", name=f"wt{j}")
                nc.scalar.dma_start(wt[:], ws[j * 128:(j + 1) * 128, :])
                nc.vector.tensor_reduce(
                    wp[:, j:j + 1], wt[:], axis=AX.X, op=AL.add,
                    apply_absolute_value=True,
                )
            nc.vector.tensor_reduce(wsum_sb[:], wp[:], axis=AX.X, op=AL.add)
            nc.scalar.dma_start(wsum_d[:], wsum_sb[:])

            # ---- section 3: collectives + global scales ----
            with nc.named_scope("coll"):
                nc.gpsimd.collective_compute(
                    "AllGather", AL.bypass,
                    replica_groups=[list(range(N_CORES))],
                    ins=[am_d[:]], outs=[am_sh[:]],
                )
                nc.gpsimd.collective_compute(
                    "AllReduce", AL.add,
                    replica_groups=[list(range(N_CORES))],
                    ins=[wsum_d[:]], outs=[wsum_sh[:]],
                )
                for b in range(N_CORES):
                    nc.sync.dma_start(
                        amg[:, b * N_OWN:(b + 1) * N_OWN],
                        am_sh[b * 128:(b + 1) * 128, :],
                    )
                nc.sync.dma_start(gsb[:], wsum_sh[:])
                nc.gpsimd.partition_all_reduce(
                    gtot[:], gsb[:], channels=128, reduce_op=bass_isa.ReduceOp.add
                )
                nc.vector.tensor_scalar(
                    wme[:], gtot[:], MEAN_SCALE, EPS, op0=AL.mult, op1=AL.max
                )
                nc.vector.reciprocal(swt[:], wme[:])
                # s = 127/max(amax, EPS); dq = max(amax,EPS) * wme / 127
                nc.vector.tensor_scalar(amg[:], amg[:], EPS, None, op0=AL.max)
                nc.vector.reciprocal(s_all[:], amg[:])
                nc.vector.tensor_scalar(s_all[:], s_all[:], 127.0, None, op0=AL.mult)
                nc.vector.tensor_scalar(
                    dq[:], amg[:], wme[:, 0:1],
                    float(np.float32(1.0 / 127.0)), op0=AL.mult, op1=AL.mult,
                )

            # ---- section 4: ternarize own slab + local transpose (resident) ----
            twT_own = twTo.tile([128, N_K, OF_PC], dt.bfloat16, tag="twTo")
            with nc.named_scope("terniarize"):
                for j in range(NWT):
                    wt2 = xpool.tile([128, D_IN], dt.float32, tag="x", name=f"wt2_{j}")
                    nc.scalar.dma_start(wt2[:], ws[j * 128:(j + 1) * 128, :])
                    nc.scalar.activation(wt2[:], wt2[:], AF.Copy, scale=swt[:, 0:1])
                    twr = qp.tile([128, D_IN], dt.bfloat16, tag="qb", name=f"twr{j}")
                    nc.vector.tensor_scalar(
                        twr[:], wt2[:], MAGIC, MAGIC, op0=AL.add, op1=AL.subtract
                    )
                    twc = qp.tile([128, D_IN], dt.bfloat16, tag="qb", name=f"twc{j}")
                    nc.vector.tensor_scalar(
                        twc[:], twr[:], 1.0, -1.0, op0=AL.min, op1=AL.max
                    )
                    nc.sync.dma_start(
                        twT_own[:, :, j * 128:(j + 1) * 128], twc[:], transpose=True
                    )

        # ---- section 5: pipelined quant + matmul over all 64 token tiles ----
        qT_tiles = [None] * N_TT

        def stage_quant(t):
            xt = xpool.tile([128, D_IN], dt.float32, tag="x", name=f"xt{t}")
            nc.scalar.dma_start(xt[:], xf[t * 128:(t + 1) * 128, :])
            nc.scalar.activation(xt[:], xt[:], AF.Copy, scale=s_all[:, t:t + 1])
            qb = qp.tile([128, D_IN], dt.bfloat16, tag="qb", name=f"qb{t}")
            nc.vector.tensor_scalar(
                qb[:], xt[:], MAGIC, MAGIC, op0=AL.add, op1=AL.subtract
            )
            qT_t = qTp.tile([128, N_K, 128], dt.bfloat16, tag="qT", name=f"qT{t}")
            nc.sync.dma_start(qT_t[:], qb[:], transpose=True)
            qT_tiles[t] = qT_t

        def stage_mm(t):
            ps = pp.tile([128, OF_PC], dt.float32, tag="ps", name=f"ps{t}")
            for k in range(N_K):
                nc.tensor.matmul(
                    ps[:], qT_tiles[t][:, k, :], twT_own[:, k, :],
                    start=(k == 0), stop=(k == N_K - 1),
                )
            ot = op.tile([128, OF_PC], dt.float32, tag="ot", name=f"ot{t}")
            nc.vector.tensor_scalar(ot[:], ps[:], dq[:, t:t + 1], None, op0=AL.mult)
            nc.gpsimd.dma_start(out[t * 128:(t + 1) * 128, :], ot[:])

        with nc.named_scope("matmul"):
            for t in range(LOOKAHEAD):
                stage_quant(t)
            for t in range(N_TT):
                if t + LOOKAHEAD < N_TT:
                    stage_quant(t + LOOKAHEAD)
                stage_mm(t)

    nc.compile()
    return nc


def _get_module():
    if "nc" not in _CACHE:
        _CACHE["nc"] = _build_module()
    return _CACHE["nc"]


def _make_in_maps(x2, w2):
    return [
        {
            "xf": x2,
            "xo": x2[i * TOK_PC:(i + 1) * TOK_PC],
            "ws": w2[i * OF_PC:(i + 1) * OF_PC],
        }
        for i in range(N_CORES)
    ]


def kernel(x: np.ndarray, weight: np.ndarray) -> np.ndarray:
    from concourse.bass_utils import run_bass_kernel_spmd

    x = np.asarray(x, dtype=np.float32)
    weight = np.asarray(weight, dtype=np.float32)
    x2 = np.ascontiguousarray(x.reshape(TOK, D_IN))
    w2 = np.ascontiguousarray(weight)

    in_maps = _make_in_maps(x2, w2)
    nc = _get_module()
    res = run_bass_kernel_spmd(nc, in_maps, list(range(N_CORES)))
    full = np.empty((TOK, D_OUT), dtype=np.float32)
    for i in range(N_CORES):
        full[:, i * OF_PC:(i + 1) * OF_PC] = res.results[i]["out"]
    return full.reshape(B, S, D_OUT)


# revision 8
# speedup vs baseline: 1.2546x; 1.2016x over previous
"""BitLinear (per-token int8 activation quant + ternary weight quant + matmul)
as a Bass/Tile kernel on 8 Trainium2 NeuronCores.

Strategy (data-parallel tokens, zero collectives):
  - x [4,2048,4096] -> [8192,4096]; each core quantizes and matmuls its own
    1024-token slab against the FULL weight; outputs concatenate on tokens.
  - Every core computes mean(|W|) itself (one 67MB streaming pass that
    overlaps activation quant), then ternarizes W slab-by-slab just in time
    for the matmul, one out_feature slab (512 cols) ahead of the PE.
    No collectives => no NRT entry barrier, no AllGather serialization.
  - q = rint(x*s) (s = 127/max(|x|) per token) and tw in {-1,0,1} are exact
    in bf16 => the bf16 matmul with fp32 PSUM accumulation is EXACT integer
    arithmetic; per-token dequant scales applied on the PSUM->SBUF copy.
  - Operand transposes (contraction on partitions) via DMA xbar SBUF->SBUF.
  - The last 4 row-blocks of the mean pass are slab 0's, kept resident so
    slab 0 ternarizes without re-reads and the matmul starts immediately
    after the mean finishes.
"""
import numpy as np
from contextlib import ExitStack

N_CORES = 8
B, S, D_IN, D_OUT = 4, 2048, 4096, 4096
TOK = B * S                  # 8192
TOK_PC = TOK // N_CORES      # 1024 tokens per core
N_TOK_TILES = TOK_PC // 128  # 8
N_K = D_IN // 128            # 32 contraction tiles
OF_CHUNK = 512
N_SLAB = D_OUT // OF_CHUNK   # 8
NWB = D_OUT // 128           # 32 weight row-blocks
EPS = 1e-5
MAGIC = float(np.float32(1.5 * 2 ** 23))   # fp32 round-to-nearest-even trick
MEAN_SCALE = float(np.float32(1.0 / (D_IN * D_OUT)))  # 2^-24, exact

_CACHE = {}


def _build_module():
    import concourse.bacc as bacc
    import concourse.tile as tile
    import concourse.mybir as mybir
    import concourse.bass_isa as bass_isa

    dt = mybir.dt
    AF = mybir.ActivationFunctionType
    AL = mybir.AluOpType
    AX = mybir.AxisListType

    nc = bacc.Bacc(
        "TRN2", target_bir_lowering=False, debug=False, num_devices=N_CORES
    )
    xs = nc.dram_tensor("xs", [TOK_PC, D_IN], dt.float32, kind="ExternalInput").ap()
    wf = nc.dram_tensor("wf", [D_OUT, D_IN], dt.float32, kind="ExternalInput").ap()
    out = nc.dram_tensor("out", [TOK_PC, D_OUT], dt.float32, kind="ExternalOutput").ap()

    with tile.TileContext(nc) as tc, ExitStack() as ctx:
        stats = ctx.enter_context(tc.tile_pool(name="stats", bufs=1))
        qT_pool = ctx.enter_context(tc.tile_pool(name="qT", bufs=N_TOK_TILES))
        big = ctx.enter_context(tc.tile_pool(name="big", bufs=3))
        qb_pool = ctx.enter_context(tc.tile_pool(name="qbp", bufs=2))
        twTp = ctx.enter_context(tc.tile_pool(name="twT", bufs=2))
        op = ctx.enter_context(tc.tile_pool(name="op", bufs=2))
        pp = ctx.enter_context(tc.tile_pool(name="pp", bufs=6, space="PSUM"))

        amc = stats.tile([128, N_TOK_TILES], dt.float32, tag="amc")
        s_all = stats.tile([128, N_TOK_TILES], dt.float32, tag="s_all")
        dq = stats.tile([128, N_TOK_TILES], dt.float32, tag="dq")
        wme = stats.tile([128, 1], dt.float32, tag="wme")
        swt = stats.tile([128, 1], dt.float32, tag="swt")
        wp = stats.tile([128, NWB], dt.float32, tag="wp")
        wsum_sb = stats.tile([128, 1], dt.float32, tag="wsum_sb")
        gtot = stats.tile([128, 1], dt.float32, tag="gtot")

        # ---- x-quant: own tokens -> resident qT tiles (sync queue) ----
        qT_tiles = []
        with nc.named_scope("xquant"):
            for t in range(N_TOK_TILES):
                xt = big.tile([128, D_IN], dt.float32, tag="big", name=f"xt{t}")
                nc.sync.dma_start(xt[:], xs[t * 128:(t + 1) * 128, :])
                nc.vector.tensor_reduce(
                    amc[:, t:t + 1], xt[:], axis=AX.X, op=AL.max,
                    apply_absolute_value=True,
                )
                nc.vector.tensor_scalar(
                    amc[:, t:t + 1], amc[:, t:t + 1], EPS, None, op0=AL.max
                )
                nc.vector.reciprocal(s_all[:, t:t + 1], amc[:, t:t + 1])
                nc.vector.tensor_scalar(
                    s_all[:, t:t + 1], s_all[:, t:t + 1], 127.0, None, op0=AL.mult
                )
                nc.scalar.activation(xt[:], xt[:], AF.Copy, scale=s_all[:, t:t + 1])
                qb = qb_pool.tile([128, D_IN], dt.bfloat16, tag="qb", name=f"qb{t}")
                nc.vector.tensor_scalar(
                    qb[:], xt[:], MAGIC, MAGIC, op0=AL.add, op1=AL.subtract
                )
                qT_t = qT_pool.tile(
                    [128, N_K, 128], dt.bfloat16, tag="qT", name=f"qT{t}"
                )
                nc.sync.dma_start(qT_t[:], qb[:], transpose=True)
                qT_tiles.append(qT_t)

        # ---- |W| mean pass: stream full W on the scalar queue ----
        # order: blocks of slabs 1..7 first, then slab 0's blocks (kept hot)
        mean_order = list(range(4, NWB)) + [0, 1, 2, 3]
        kept = {}
        with nc.named_scope("wmean"):
            for j in mean_order:
                wt = big.tile([128, D_IN], dt.float32, tag="big", name=f"wm{j}")
                nc.scalar.dma_start(wt[:], wf[j * 128:(j + 1) * 128, :])
                nc.vector.tensor_reduce(
                    wp[:, j:j + 1], wt[:], axis=AX.X, op=AL.add,
                    apply_absolute_value=True,
                )
                if j < 4:
                    kept[j] = wt
            nc.vector.tensor_reduce(wsum_sb[:], wp[:], axis=AX.X, op=AL.add)
            nc.gpsimd.partition_all_reduce(
                gtot[:], wsum_sb[:], channels=128, reduce_op=bass_isa.ReduceOp.add
            )
            nc.vector.tensor_scalar(
                wme[:], gtot[:], MEAN_SCALE, EPS, op0=AL.mult, op1=AL.max
            )
            nc.vector.reciprocal(swt[:], wme[:])
            for t in range(N_TOK_TILES):
                nc.vector.tensor_scalar(
                    dq[:, t:t + 1], amc[:, t:t + 1], wme[:, 0:1],
                    float(np.float32(1.0 / 127.0)), op0=AL.mult, op1=AL.mult,
                )

        # ---- per-slab: ternarize+transpose one slab ahead, then matmul ----
        def stage_tern(c):
            twT_c = twTp.tile(
                [128, N_K, OF_CHUNK], dt.bfloat16, tag="twT", name=f"twT{c}"
            )
            # process row-blocks; for slab 0 reuse the kept mean tiles
            order = [1, 2, 3, 0] if c == 0 else [0, 1, 2, 3]
            for j in order:
                blk = 4 * c + j
                if c == 0 and j in (1, 2, 3):
                    wt = kept[j]
                else:
                    wt = big.tile(
                        [128, D_IN], dt.float32, tag="big", name=f"wt{blk}"
                    )
                    nc.scalar.dma_start(wt[:], wf[blk * 128:(blk + 1) * 128, :])
                nc.scalar.activation(wt[:], wt[:], AF.Copy, scale=swt[:, 0:1])
                twr = qb_pool.tile([128, D_IN], dt.bfloat16, tag="qb", name=f"twr{blk}")
                nc.vector.tensor_scalar(
                    twr[:], wt[:], MAGIC, MAGIC, op0=AL.add, op1=AL.subtract
                )
                twc = qb_pool.tile([128, D_IN], dt.bfloat16, tag="qb", name=f"twc{blk}")
                nc.vector.tensor_scalar(
                    twc[:], twr[:], 1.0, -1.0, op0=AL.min, op1=AL.max
                )
                nc.sync.dma_start(
                    twT_c[:, :, j * 128:(j + 1) * 128], twc[:], transpose=True
                )
            return twT_c

        def stage_mm(c, twT_c):
            for t in range(N_TOK_TILES):
                ps = pp.tile([128, OF_CHUNK], dt.float32, tag="ps", name=f"ps{c}_{t}")
                for k in range(N_K):
                    nc.tensor.matmul(
                        ps[:], qT_tiles[t][:, k, :], twT_c[:, k, :],
                        start=(k == 0), stop=(k == N_K - 1),
                    )
                ot = op.tile([128, OF_CHUNK], dt.float32, tag="ot", name=f"ot{c}_{t}")
                nc.vector.tensor_scalar(
                    ot[:], ps[:], dq[:, t:t + 1], None, op0=AL.mult
                )
                nc.gpsimd.dma_start(
                    out[t * 128:(t + 1) * 128, c * OF_CHUNK:(c + 1) * OF_CHUNK],
                    ot[:],
                )

        with nc.named_scope("mm"):
            twT_cur = stage_tern(0)
            for c in range(N_SLAB):
                twT_next = stage_tern(c + 1) if c + 1 < N_SLAB else None
                stage_mm(c, twT_cur)
                twT_cur = twT_next

    nc.compile()
    return nc


def _get_module():
    if "nc" not in _CACHE:
        _CACHE["nc"] = _build_module()
    return _CACHE["nc"]


def _make_in_maps(x2, w2):
    return [
        {
            "xs": x2[i * TOK_PC:(i + 1) * TOK_PC],
            "wf": w2,
        }
        for i in range(N_CORES)
    ]


def kernel(x: np.ndarray, weight: np.ndarray) -> np.ndarray:
    from concourse.bass_utils import run_bass_kernel_spmd

    x = np.asarray(x, dtype=np.float32)
    weight = np.asarray(weight, dtype=np.float32)
    x2 = np.ascontiguousarray(x.reshape(TOK, D_IN))
    w2 = np.ascontiguousarray(weight)

    in_maps = _make_in_maps(x2, w2)
    nc = _get_module()
    res = run_bass_kernel_spmd(nc, in_maps, list(range(N_CORES)))
    out = np.concatenate([res.results[i]["out"] for i in range(N_CORES)], axis=0)
    return out.reshape(B, S, D_OUT)
